# revision 1
# baseline (speedup 1.0000x reference)
"""ChannelSA Trainium2 kernel: 8-way batch-parallel across NeuronCores.

kernel(**inputs) takes the FULL inputs (x [8,192,128,128], conv weights,
pos_emb) and returns the FULL output [8,192,128,128] fp32. Each core runs
an identical single-batch program (SPMD, no collectives).

Per-core pipeline:
  z   = W1 @ x                   1x1 conv, fp32r matmuls (K=192 -> 128+64)
  qkv = DW3x3(z)                 9 accumulating diagonal matmuls on TensorE
                                 over a zero-padded bf16 z layout (shifted APs)
  q,k -> bf16 DMA-transpose ->   per-head Gram banks [Gqk|Gqq|Gkk] in PSUM
  logits = Gqk / (|q||k| sqrt(48))   norms taken from the Gram diagonals;
                                 pos_emb is constant per softmax row: a no-op
  attn = softmax(logits)
  y = (W_out @ blockdiag(attn)) @ v   output projection fused with attn@v
"""
import math
from contextlib import ExitStack

import numpy as np

import concourse.bass as bass
import concourse.mybir as mybir
import concourse.tile as tile
from concourse.masks import make_identity

F32 = mybir.dt.float32
F32R = mybir.dt.float32r
BF16 = mybir.dt.bfloat16
AF = mybir.ActivationFunctionType

C = 192
CQKV = 576
H = 128
W = 128
L = H * W
HEADS = 4
DH = 48
R = 8                    # output image rows per chunk
NCHUNK = H // R
PADW = W + 2             # padded row stride in z tiles
ZROWS = R + 2            # rows held per z chunk (1 halo each side)
TAPS = [(di, dj) for di in (-1, 0, 1) for dj in (-1, 0, 1)]
BLKS = [(0, 128), (128, 256), (256, 384), (384, 512), (512, 576)]
N_CORES = 8

_MAX_DRAIN_WAITS = 1


def _patch_tail_drain():
    """The walrus in this image rejects >1 semaphore wait on the Tile tail
    drain instruction; split the waits across a chain of SP nops."""
    if getattr(tile.TileContext, "_drain_patched", False):
        return

    def _drain_and_barrier(self, tick_clock, wait_clock):
        from concourse.vector_clock import ScopedClock

        nc = self.nc
        drain_inst = nc.sync.drain()
        wait_clock.add_sem_waits(
            drain_inst.ins, ScopedClock({None: tick_clock.global_clock})
        )
        si = drain_inst.ins.sync_info
        waits = list(si.on_wait or [])
        if len(waits) > _MAX_DRAIN_WAITS:
            si.on_wait = waits[:_MAX_DRAIN_WAITS]
            rest = waits[_MAX_DRAIN_WAITS:]
            for i in range(0, len(rest), _MAX_DRAIN_WAITS):
                nop = nc.sync.nop(nofuse=True)
                nop.ins.sync_info = mybir.SyncInfo(
                    on_wait=rest[i : i + _MAX_DRAIN_WAITS], on_update=[]
                )
        nc.all_engine_barrier()
        assert self.sems is not None
        popped = nc._tile_sem_poison_stack.pop()
        assert popped is self._sem_poison
        nc.clear_and_free_semaphores(list(self.sems.allocated().values()))
        nc.all_engine_barrier()

    tile.TileContext._drain_and_barrier = _drain_and_barrier
    tile.TileContext._drain_patched = True


def build_nc(split_waits=True):
    _patch_tail_drain()
    nc = bass.Bass("TRN2", target_bir_lowering=False, debug=False)

    x_d = nc.declare_dram_parameter("x", [C, L], BF16, isOutput=False)
    w1t_d = nc.declare_dram_parameter("w1t", [C, CQKV], BF16, isOutput=False)
    wdw_d = nc.declare_dram_parameter("wdw", [CQKV, 9], F32, isOutput=False)
    woutt_d = nc.declare_dram_parameter("woutt", [C, C], F32, isOutput=False)
    y_d = nc.declare_dram_parameter("y", [C, L], F32, isOutput=True)

    with tile.TileContext(nc) as tc, ExitStack() as ctx:
        _body(ctx, tc, x_d, w1t_d, wdw_d, woutt_d, y_d)
    if split_waits:
        # CoreSim can't run the split module (its race detector wants sem
        # updates on every inst); the split is only needed for walrus.
        _split_excess_waits(nc)
    return nc


def _split_excess_waits(nc, maxw=1):
    """This walrus build accepts only one semaphore wait per instruction.
    Move excess waits onto same-engine no-ops inserted just before the
    offending instruction (same-engine program order preserves semantics)."""
    uid = [0]
    for f in nc.m.functions:
        for bb in f.blocks:
            il = bb.instructions
            out = []
            changed = False
            for inst in il:
                si = inst.sync_info
                waits = list(si.on_wait) if si and si.on_wait else []
                if len(waits) > maxw:
                    changed = True
                    rest, keep = waits[:-maxw], waits[-maxw:]
                    for i in range(0, len(rest), maxw):
                        uid[0] += 1
                        out.append(
                            mybir.InstNoOp(
                                name=f"I-waitsplit-{uid[0]}",
                                engine=inst.engine,
                                ins=[],
                                outs=[],
                                sync_info=mybir.SyncInfo(
                                    on_wait=rest[i : i + maxw], on_update=[]
                                ),
                            )
                        )
                    si.on_wait = keep
                out.append(inst)
            if changed:
                bb.instructions = out


def _body(ctx, tc, x_d, w1t_d, wdw_d, woutt_d, y_d):
    nc = tc.nc
    ncopy = [0]

    def copy(dst, src):
        # alternate PSUM->SBUF copies between ACT and DVE
        if ncopy[0] % 2 == 0:
            nc.scalar.copy(dst, src)
        else:
            nc.vector.tensor_copy(dst, src)
        ncopy[0] += 1

    const = ctx.enter_context(tc.tile_pool(name="const", bufs=1))
    persist = ctx.enter_context(tc.tile_pool(name="persist", bufs=1))

    # ---- constants / weights ----
    # K- and M-padded conv1 weights: rows 64:128 of w1t1 and cols 576:640
    # of both are zero so every conv1 matmul is a full 128x128 pass
    w1t0 = const.tile([128, 640], BF16, tag="w1t0")
    w1t1 = const.tile([128, 640], BF16, tag="w1t1")
    nc.gpsimd.memset(w1t0[:], 0.0)
    nc.gpsimd.memset(w1t1[:], 0.0)
    nc.sync.dma_start(w1t0[:, 0:CQKV], w1t_d[0:128, :])
    nc.sync.dma_start(w1t1[0:64, 0:CQKV], w1t_d[128:192, :])

    woutt0 = const.tile([128, C], F32, tag="woutt0")
    woutt1 = const.tile([64, C], F32, tag="woutt1")
    nc.sync.dma_start(woutt0[:], woutt_d[0:128, :])
    nc.sync.dma_start(woutt1[:], woutt_d[128:192, :])
    woutt0_bf = const.tile([128, C], BF16, tag="woutt0bf")
    woutt1_bf = const.tile([64, C], BF16, tag="woutt1bf")
    nc.vector.tensor_copy(woutt0_bf[:], woutt0[:])
    nc.vector.tensor_copy(woutt1_bf[:], woutt1[:])

    ident_bf = const.tile([128, 128], BF16, tag="identbf")
    make_identity(nc, ident_bf[:])
    ident48 = const.tile([48, 48], F32, tag="ident48")
    make_identity(nc, ident48[:])
    ones48 = const.tile([48, 1], F32, tag="ones48")
    nc.gpsimd.memset(ones48[:], 1.0)
    ones1x48 = const.tile([1, 48], F32, tag="ones1x48")
    nc.gpsimd.memset(ones1x48[:], 1.0)

    # dw weights -> 45 diagonal bf16 matrices
    diagw = []
    for b, (c0, c1) in enumerate(BLKS):
        p = c1 - c0
        wdw_sb = const.tile([p, 9], F32, tag=f"wdw{b}")
        nc.sync.dma_start(wdw_sb[:], wdw_d[c0:c1, :])
        row = []
        for t in range(9):
            dt_ = const.tile([p, p], BF16, tag=f"diag{b}_{t}")
            nc.vector.tensor_scalar_mul(dt_[:], ident_bf[0:p, 0:p], wdw_sb[:, t : t + 1])
            row.append(dt_)
        diagw.append(row)

    # ---- persistent state ----
    v0 = persist.tile([128, L], BF16, tag="v0")
    v1 = persist.tile([128, L], BF16, tag="v1")
    nc.gpsimd.memset(v1[64:128, :], 0.0)
    zt = [
        [
            persist.tile([c1 - c0, ZROWS, PADW], BF16, tag=f"z{s}_{b}", name=f"z{s}_{b}")
            for b, (c0, c1) in enumerate(BLKS)
        ]
        for s in range(2)
    ]
    for s in range(2):
        for b in range(5):
            nc.gpsimd.memset(zt[s][b][:], 0.0)

    ghs = persist.tile([48, HEADS * 144], F32, tag="ghs")
    xt1_pp = [persist.tile([128, ZROWS, W], BF16, tag=f"xt1_{s}", name=f"xt1_{s}") for s in range(2)]
    for s in range(2):
        nc.gpsimd.memset(xt1_pp[s][:], 0.0)

    # ---- phase A: chunked pipeline ----
    with (
        tc.tile_pool(name="gps", bufs=1, space="PSUM") as gps,
        tc.tile_pool(name="xp", bufs=2) as xp,
        tc.tile_pool(name="zps", bufs=3, space="PSUM") as zps,
        tc.tile_pool(name="qps", bufs=3, space="PSUM") as qps,
        tc.tile_pool(name="stp", bufs=2) as stp,
        tc.tile_pool(name="qktp", bufs=2) as qktp,
    ):
        # two G banks; a single accumulation group spans all heads per bank
        # (only the globally-first matmul into each bank carries start=True)
        g1 = gps.tile([48, HEADS * 96], F32, tag="g1")
        g2 = gps.tile([48, HEADS * 48], F32, tag="g2")
        for c in range(NCHUNK):
            zs = zt[c % 2]
            r0 = max(0, R * c - 1)
            r1 = min(H, R * c + R + 1)
            nrows = r1 - r0
            brow0 = r0 - (R * c - 1)  # buf row of image row r0

            xt0 = xp.tile([128, nrows, W], BF16, tag="x0")
            xt1 = xt1_pp[c % 2]
            nc.sync.dma_start(
                xt0[:], x_d[0:128, r0 * W : r1 * W].rearrange("p (r w) -> p r w", w=W)
            )
            nc.sync.dma_start(
                xt1[0:64, 0:nrows, :],
                x_d[128:192, r0 * W : r1 * W].rearrange("p (r w) -> p r w", w=W),
            )

            # conv1 into padded z tiles (groups of <=4 rows)
            for g0 in range(0, nrows, 4):
                gn = min(4, nrows - g0)
                for b, (c0, c1) in enumerate(BLKS):
                    p = c1 - c0
                    ps = zps.tile([128, 512], F32, tag="zps")
                    nc.tensor.matmul(
                        ps[:, 0 : gn * W],
                        w1t0[:, c0 : c0 + 128],
                        xt0[:, g0 : g0 + gn, :],
                        start=True,
                        stop=False,
                    )
                    nc.tensor.matmul(
                        ps[:, 0 : gn * W],
                        w1t1[:, c0 : c0 + 128],
                        xt1[:, g0 : g0 + gn, :],
                        start=False,
                        stop=True,
                    )
                    copy(zs[b][:, brow0 + g0 : brow0 + g0 + gn, 1 : 1 + W], ps[0:p, 0 : gn * W])

            if c == NCHUNK - 1:
                # bottom halo row never written this chunk; clear stale data
                for b in range(5):
                    nc.gpsimd.memset(zs[b][:, ZROWS - 1 : ZROWS, :], 0.0)

            # taps: 9 accumulating diagonal matmuls -> qkv rows Rc..Rc+R
            st = [stp.tile([128, R // 4, 4 * W], BF16, tag=f"st{i}", name=f"st{i}") for i in range(3)]
            for g in range(R // 4):
                orow = 1 + 4 * g  # buf row of first output row in this group
                for b, (c0, c1) in enumerate(BLKS):
                    p = c1 - c0
                    ps = qps.tile([128, 512], F32, tag="qps")
                    for t, (di, dj) in enumerate(TAPS):
                        nc.tensor.matmul(
                            ps[0:p, :],
                            diagw[b][t][:],
                            zs[b][:, orow + di : orow + di + 4, 1 + dj : 1 + dj + W],
                            start=(t == 0),
                            stop=(t == 8),
                        )
                    if b < 3:
                        copy(st[b][:, g, :], ps[:, :])
                    elif b == 3:
                        copy(v0[:, c * R * W + g * 512 : c * R * W + (g + 1) * 512], ps[:, :])
                    else:
                        copy(v1[0:64, c * R * W + g * 512 : c * R * W + (g + 1) * 512], ps[0:64, :])

            # transpose q,k: qkt[:, lt, 0, :] = k^T, [:, lt, 1, :] = q^T
            # batched 3D-out form: out[p, lt, c] = in[c, lt*128 + p]
            st_flat = [s.rearrange("p a b -> p (a b)") for s in st]
            qkt = qktp.tile([128, R, 2, 192], BF16, tag="qkt")
            nc.sync.dma_start_transpose(qkt[:, :, 1, 0:128], st_flat[0][:, :])
            nc.scalar.dma_start_transpose(qkt[:, :, 1, 128:192], st_flat[1][0:64, :])
            nc.sync.dma_start_transpose(qkt[:, :, 0, 0:64], st_flat[1][64:128, :])
            nc.scalar.dma_start_transpose(qkt[:, :, 0, 64:192], st_flat[2][:, :])

            # gram accumulation
            for lt in range(R):
                first = c == 0 and lt == 0
                last = c == NCHUNK - 1 and lt == R - 1
                for h in range(HEADS):
                    nc.tensor.matmul(
                        g1[:, h * 96 : h * 96 + 96],
                        qkt[:, lt, 1, h * DH : (h + 1) * DH],
                        qkt[:, lt, :, h * DH : (h + 1) * DH],
                        start=(first and h == 0),
                        stop=(last and h == HEADS - 1),
                        skip_group_check=True,
                    )
                    nc.tensor.matmul(
                        g2[:, h * DH : (h + 1) * DH],
                        qkt[:, lt, 0, h * DH : (h + 1) * DH],
                        qkt[:, lt, 0, h * DH : (h + 1) * DH],
                        start=(first and h == 0),
                        stop=(last and h == HEADS - 1),
                        skip_group_check=True,
                    )

        nc.vector.tensor_copy(ghs[:, 0 : HEADS * 96], g1[:])
        nc.vector.tensor_copy(ghs[:, HEADS * 96 :], g2[:])

    # ---- phase B ----
    with (
        tc.tile_pool(name="bsb", bufs=1) as bsb,
        tc.tile_pool(name="bps", bufs=1, space="PSUM") as bps,
        tc.tile_pool(name="ops", bufs=4, space="PSUM") as ops,
        tc.tile_pool(name="osb", bufs=4) as osb,
    ):
        attn_bf = bsb.tile([48, HEADS * 48], BF16, tag="attnbf")
        scr = bsb.tile([48, 48], F32, tag="scr")
        scr2 = bsb.tile([48, 48], F32, tag="scr2")
        colv = bsb.tile([48, 1], F32, tag="colv")
        rowv = bsb.tile([1, 48], F32, tag="rowv")
        rkrep = bsb.tile([48, 48], F32, tag="rkrep")
        logits = bsb.tile([48, 48], F32, tag="logits")

        for h in range(HEADS):
            gqk = ghs[:, h * 96 : h * 96 + 48]
            gqq = ghs[:, h * 96 + 48 : h * 96 + 96]
            gkk = ghs[:, HEADS * 96 + h * DH : HEADS * 96 + (h + 1) * DH]

            # rq_inv = 1/max(sqrt(diag(Gqq)),eps), with 1/sqrt(DH) folded in
            nc.vector.tensor_mul(scr[:], gqq, ident48[:])
            nc.vector.reduce_sum(colv[:], scr[:], axis=mybir.AxisListType.X)
            nc.scalar.activation(colv[:], colv[:], AF.Sqrt)
            nc.vector.tensor_scalar_max(colv[:], colv[:], 1e-12)
            nc.vector.reciprocal(colv[:], colv[:])
            nc.vector.tensor_scalar(
                logits[:],
                gqk,
                colv[:],
                1.0 / math.sqrt(DH),
                op0=mybir.AluOpType.mult,
                op1=mybir.AluOpType.mult,
            )

            # rk_inv broadcast along the free (key) dim via diag-as-row
            nc.vector.tensor_mul(scr2[:], gkk, ident48[:])
            ps_row = bps.tile([1, 48], F32, tag="pssmall")
            nc.tensor.matmul(ps_row[:], ones48[:], scr2[:], start=True, stop=True)
            nc.vector.tensor_copy(rowv[:], ps_row[:])
            nc.scalar.activation(rowv[:], rowv[:], AF.Sqrt)
            nc.vector.tensor_scalar_max(rowv[:], rowv[:], 1e-12)
            nc.vector.reciprocal(rowv[:], rowv[:])
            ps_rep = bps.tile([48, 48], F32, tag="pssmall")
            nc.tensor.matmul(ps_rep[:], ones1x48[:], rowv[:], start=True, stop=True)
            nc.vector.tensor_copy(rkrep[:], ps_rep[:])
            nc.vector.tensor_mul(logits[:], logits[:], rkrep[:])

            # softmax over the free (key) dim
            nc.vector.reduce_max(colv[:], logits[:], axis=mybir.AxisListType.X)
            nc.vector.tensor_scalar_sub(logits[:], logits[:], colv[:])
            nc.scalar.activation(logits[:], logits[:], AF.Exp)
            nc.vector.reduce_sum(colv[:], logits[:], axis=mybir.AxisListType.X)
            nc.vector.reciprocal(colv[:], colv[:])
            nc.vector.tensor_scalar_mul(logits[:], logits[:], colv[:])
            nc.vector.tensor_copy(attn_bf[:, h * 48 : (h + 1) * 48], logits[:])

        # block-diagonal attn (bf16)
        bd0 = bsb.tile([128, C], BF16, tag="bd0")
        bd1 = bsb.tile([64, C], BF16, tag="bd1")
        nc.gpsimd.memset(bd0[:], 0.0)
        nc.gpsimd.memset(bd1[:], 0.0)
        nc.sync.dma_start(bd0[0:48, 0:48], attn_bf[:, 0:48])
        nc.sync.dma_start(bd0[48:96, 48:96], attn_bf[:, 48:96])
        nc.sync.dma_start(bd0[96:128, 96:144], attn_bf[0:32, 96:144])
        nc.sync.dma_start(bd1[0:16, 96:144], attn_bf[32:48, 96:144])
        nc.sync.dma_start(bd1[16:64, 144:192], attn_bf[:, 144:192])

        # W_effT = BD(attn).T @ W_outT   [192 x 192], bf16
        weff0 = bsb.tile([128, 256], BF16, tag="weff0")
        weff1 = bsb.tile([128, 256], BF16, tag="weff1")
        nc.gpsimd.memset(weff0[:], 0.0)
        nc.gpsimd.memset(weff1[:], 0.0)
        for m0, m1, wt in [(0, 128, weff0), (128, 192, weff1)]:
            pw = bps.tile([128, C], F32, tag="pweff")
            nc.tensor.matmul(pw[0 : m1 - m0, :], bd0[:, m0:m1], woutt0_bf[:], start=True, stop=False)
            nc.tensor.matmul(pw[0 : m1 - m0, :], bd1[:, m0:m1], woutt1_bf[:], start=False, stop=True)
            copy(wt[0 : m1 - m0, 0:C], pw[0 : m1 - m0, :])

        # y = W_effT.T @ v
        for g in range(L // 512):
            sl = slice(g * 512, (g + 1) * 512)
            for m0, m1 in [(0, 128), (128, 192)]:
                po = ops.tile([128, 512], F32, tag="ops")
                nc.tensor.matmul(po[:, :], weff0[:, m0 : m0 + 128], v0[:, sl], start=True, stop=False)
                nc.tensor.matmul(po[:, :], weff1[:, m0 : m0 + 128], v1[:, sl], start=False, stop=True)
                ot = osb.tile([m1 - m0, 512], F32, tag=f"o{m0}", name=f"o{m0}")
                copy(ot[:], po[0 : m1 - m0, :])
                nc.sync.dma_start(y_d[m0:m1, sl], ot[:])


_NC_CACHE = None


def _get_nc():
    global _NC_CACHE
    if _NC_CACHE is None:
        _NC_CACHE = build_nc()
    return _NC_CACHE


def kernel(x, w_proj1, w_dw, pos_emb, w_out, _trace=False):
    from concourse.bass_utils import run_bass_kernel_spmd

    import ml_dtypes

    x = np.asarray(x, dtype=np.float32).astype(ml_dtypes.bfloat16)
    w1t = np.ascontiguousarray(
        np.asarray(w_proj1, np.float32).reshape(CQKV, C).T.astype(ml_dtypes.bfloat16)
    )
    wdw = np.ascontiguousarray(np.asarray(w_dw, np.float32).reshape(CQKV, 9))
    woutt = np.ascontiguousarray(np.asarray(w_out, np.float32).reshape(C, C).T)
    # pos_emb adds a per-head constant to every logit in its softmax row;
    # softmax is shift-invariant, so it has no effect on the output.

    nc = _get_nc()
    in_maps = [
        {"x": np.ascontiguousarray(x[b].reshape(C, L)), "w1t": w1t, "wdw": wdw, "woutt": woutt}
        for b in range(N_CORES)
    ]
    res = run_bass_kernel_spmd(nc, in_maps, list(range(N_CORES)), trace=_trace)
    out = np.stack([res.results[b]["y"].reshape(C, H, W) for b in range(N_CORES)])
    if _trace:
        kernel.last_exec_time_ns = res.exec_time_ns
        kernel.last_profile = res
    return out.astype(np.float32)



# revision 11
# speedup vs baseline: 1.1042x; 1.1042x over previous
"""ChannelSA Trainium2 kernel: 8-way batch-parallel across NeuronCores.

kernel(**inputs) takes the FULL inputs (x [8,192,128,128], conv weights,
pos_emb) and returns the FULL output [8,192,128,128] fp32. Each core runs
an identical single-batch program (SPMD, no collectives).

Per-core pipeline (v2 — fp8 DoubleRow for the q/k path):
  z_qk = W1qk @ x            fp8e4 DoubleRow matmuls (K=192 in ONE pass)
  z_v  = W1v @ x             bf16 (K=128+64), v-remainder cols duplicated
  qk   = DW3x3(z_qk)         5 accumulating fp8 DoubleRow diag matmuls per
                             block (tap pairs via strided moving APs over a
                             zero-padded flat (rows x PADW) run)
  v    = DW3x3(z_v)          9 bf16 diag matmuls; 64-chan remainder packed
                             two-spatial-groups-per-pass into K=128
  q,k -> bf16 DMA-transpose -> per-head [q|k]^T[q|k] single-matmul Grams
  logits = Gqk/(|q||k| sqrt(48)); pos_emb const per row: no-op in softmax
  attn = softmax(logits)
  y = (W_out @ blockdiag(attn)) @ v   output projection fused with attn@v
"""
import math
from contextlib import ExitStack

import numpy as np

import bass_rust
import concourse.bass as bass
import concourse.mybir as mybir
import concourse.tile as tile
from concourse.masks import make_identity

F32 = mybir.dt.float32
BF16 = mybir.dt.bfloat16
FP8 = mybir.dt.float8e4
AF = mybir.ActivationFunctionType
DR = mybir.MatmulPerfMode.DoubleRow

C = 192
CQKV = 576
H = 128
W = 128
L = H * W
HEADS = 4
DH = 48
R = 8                    # output image rows per chunk
NCHUNK = H // R
PADW = W + 2             # padded row stride in z tiles
ZROWS = R + 2            # rows held per z chunk (1 halo each side)
N_CORES = 8

# DoubleRow tap pairs for the 3x3 depthwise conv: (tap_a, tap_b); None = zero
PAIRS = [
    ((-1, -1), (-1, 1)),
    ((0, -1), (0, 1)),
    ((1, -1), (1, 1)),
    ((-1, 0), (1, 0)),
    ((0, 0), None),
]
TAPS = [(di, dj) for di in (-1, 0, 1) for dj in (-1, 0, 1)]

_MAX_DRAIN_WAITS = 1


def _view(ap, dims, extra_off=0):
    return bass_rust.AP(ap.tensor, ap.offset + extra_off, dims)


def _patch_tail_drain():
    """The walrus in this image rejects >1 semaphore wait on the Tile tail
    drain instruction; split the waits across a chain of SP nops."""
    if getattr(tile.TileContext, "_drain_patched", False):
        return

    def _drain_and_barrier(self, tick_clock, wait_clock):
        from concourse.vector_clock import ScopedClock

        nc = self.nc
        drain_inst = nc.sync.drain()
        wait_clock.add_sem_waits(
            drain_inst.ins, ScopedClock({None: tick_clock.global_clock})
        )
        si = drain_inst.ins.sync_info
        waits = list(si.on_wait or [])
        if len(waits) > _MAX_DRAIN_WAITS:
            si.on_wait = waits[:_MAX_DRAIN_WAITS]
            rest = waits[_MAX_DRAIN_WAITS:]
            for i in range(0, len(rest), _MAX_DRAIN_WAITS):
                nop = nc.sync.nop(nofuse=True)
                nop.ins.sync_info = mybir.SyncInfo(
                    on_wait=rest[i : i + _MAX_DRAIN_WAITS], on_update=[]
                )
        nc.all_engine_barrier()
        assert self.sems is not None
        popped = nc._tile_sem_poison_stack.pop()
        assert popped is self._sem_poison
        nc.clear_and_free_semaphores(list(self.sems.allocated().values()))
        nc.all_engine_barrier()

    tile.TileContext._drain_and_barrier = _drain_and_barrier
    tile.TileContext._drain_patched = True


def build_nc(split_waits=True):
    _patch_tail_drain()
    nc = bass.Bass("TRN2", target_bir_lowering=False, debug=False)

    xil_d = nc.declare_dram_parameter("xil", [128, 2 * L], FP8, isOutput=False)
    xbf_d = nc.declare_dram_parameter("xbf", [C, L], BF16, isOutput=False)
    w1qk_d = nc.declare_dram_parameter("w1qk", [128, 2 * 384], FP8, isOutput=False)
    w1v_d = nc.declare_dram_parameter("w1v", [C, 256], BF16, isOutput=False)
    wdw_d = nc.declare_dram_parameter("wdw", [640, 9], F32, isOutput=False)
    woutt_d = nc.declare_dram_parameter("woutt", [C, C], BF16, isOutput=False)
    y_d = nc.declare_dram_parameter("y", [C, L], F32, isOutput=True)

    with tile.TileContext(nc) as tc, ExitStack() as ctx:
        _body(ctx, tc, xil_d, xbf_d, w1qk_d, w1v_d, wdw_d, woutt_d, y_d)
    if split_waits:
        # CoreSim can't run the split module (its race detector wants sem
        # updates on every inst); the split is only needed for walrus.
        _split_excess_waits(nc)
    return nc


def _split_excess_waits(nc, maxw=1):
    """This walrus build accepts only one semaphore wait per instruction.
    Move excess waits onto same-engine no-ops inserted just before the
    offending instruction (same-engine program order preserves semantics)."""
    uid = [0]
    for f in nc.m.functions:
        for bb in f.blocks:
            il = bb.instructions
            out = []
            changed = False
            for inst in il:
                si = inst.sync_info
                waits = list(si.on_wait) if si and si.on_wait else []
                if len(waits) > maxw:
                    changed = True
                    rest, keep = waits[:-maxw], waits[-maxw:]
                    for i in range(0, len(rest), maxw):
                        uid[0] += 1
                        out.append(
                            mybir.InstNoOp(
                                name=f"I-waitsplit-{uid[0]}",
                                engine=inst.engine,
                                ins=[],
                                outs=[],
                                sync_info=mybir.SyncInfo(
                                    on_wait=rest[i : i + maxw], on_update=[]
                                ),
                            )
                        )
                    si.on_wait = keep
                out.append(inst)
            if changed:
                bb.instructions = out


def _body(ctx, tc, xil_d, xbf_d, w1qk_d, w1v_d, wdw_d, woutt_d, y_d):
    nc = tc.nc
    ncopy = [0]

    def copy(dst, src):
        # alternate PSUM->SBUF copies between ACT and DVE
        if ncopy[0] % 2 == 0:
            nc.scalar.copy(dst, src)
        else:
            nc.vector.tensor_copy(dst, src)
        ncopy[0] += 1

    const = ctx.enter_context(tc.tile_pool(name="const", bufs=1))
    persist = ctx.enter_context(tc.tile_pool(name="persist", bufs=1))

    # ---- constants / weights ----
    w1qk = const.tile([128, 2, 384], FP8, tag="w1qk")
    nc.sync.dma_start(
        w1qk[:], w1qk_d[:].rearrange("p (i m) -> p i m", i=2)
    )
    w1v = const.tile([128, 256], BF16, tag="w1vlo")
    w1vh = const.tile([64, 256], BF16, tag="w1vhi")
    nc.sync.dma_start(w1v[:], w1v_d[0:128, :])
    nc.sync.dma_start(w1vh[:], w1v_d[128:192, :])

    woutt0_bf = const.tile([128, C], BF16, tag="woutt0bf")
    woutt1_bf = const.tile([64, C], BF16, tag="woutt1bf")
    nc.sync.dma_start(woutt0_bf[:], woutt_d[0:128, :])
    nc.sync.dma_start(woutt1_bf[:], woutt_d[128:192, :])

    ident8 = const.tile([128, 128], FP8, tag="ident8")
    make_identity(nc, ident8[:])
    ident_bf = const.tile([128, 128], BF16, tag="identbf")
    make_identity(nc, ident_bf[:])
    ident48 = const.tile([48, 48], F32, tag="ident48")
    make_identity(nc, ident48[:])
    ones1x48 = const.tile([1, 48], F32, tag="ones1x48")
    nc.gpsimd.memset(ones1x48[:], 1.0)
    ones128 = const.tile([128, 1], F32, tag="ones128")
    nc.gpsimd.memset(ones128[:], 1.0)
    # I48 block parked at partitions 64:112 (for diag(Gkk) extraction)
    ident48b = const.tile([128, 48], F32, tag="ident48b")
    nc.gpsimd.memset(ident48b[:], 0.0)
    nc.sync.dma_start(ident48b[64:112, :], ident48[:])

    # dw weights: fp8 DoubleRow pair-stationaries for q/k blocks
    dwstat = []
    for b in range(3):
        wdw_sb = const.tile([128, 9], F32, tag=f"wdwqk{b}")
        nc.sync.dma_start(wdw_sb[:], wdw_d[b * 128 : (b + 1) * 128, :])
        row = []
        for pi, (ta, tb) in enumerate(PAIRS):
            st = const.tile([128, 2, 128], FP8, tag=f"dwst{b}_{pi}")
            ti = (ta[0] + 1) * 3 + (ta[1] + 1)
            nc.vector.tensor_scalar_mul(st[:, 0, :], ident8[:], wdw_sb[:, ti : ti + 1])
            if tb is None:
                nc.gpsimd.memset(st[:, 1, :], 0.0)
            else:
                ti = (tb[0] + 1) * 3 + (tb[1] + 1)
                nc.vector.tensor_scalar_mul(
                    st[:, 1, :], ident8[:], wdw_sb[:, ti : ti + 1]
                )
            row.append(st)
        dwstat.append(row)

    # bf16 diag stationaries for the v blocks (wdw rows 384:512 and the
    # duplicated 512:576 pair shipped at rows 512:640)
    diagv = []
    for b, r0w in enumerate((384, 512)):
        wdw_sb = const.tile([128, 9], F32, tag=f"wdwv{b}")
        nc.sync.dma_start(wdw_sb[:], wdw_d[r0w : r0w + 128, :])
        row = []
        for t in range(9):
            dt_ = const.tile([128, 128], BF16, tag=f"diagv{b}_{t}")
            nc.vector.tensor_scalar_mul(dt_[:], ident_bf[:], wdw_sb[:, t : t + 1])
            row.append(dt_)
        diagv.append(row)

    # ---- persistent state ----
    v0 = persist.tile([128, L], BF16, tag="v0")
    v1 = persist.tile([128, L // 2], BF16, tag="v1")
    zqk = [
        [
            persist.tile(
                [128, ZROWS, PADW], FP8, tag=f"zqk{s}_{b}", name=f"zqk{s}_{b}"
            )
            for b in range(3)
        ]
        for s in range(2)
    ]
    zv0 = [
        persist.tile([128, ZROWS, PADW], BF16, tag=f"zv0_{s}", name=f"zv0_{s}")
        for s in range(2)
    ]
    zv1p = [
        persist.tile([128, 6, PADW], BF16, tag=f"zv1p_{s}", name=f"zv1p_{s}")
        for s in range(2)
    ]
    for s in range(2):
        for b in range(3):
            nc.gpsimd.memset(zqk[s][b][:], 0.0)
        nc.gpsimd.memset(zv0[s][:], 0.0)
        nc.gpsimd.memset(zv1p[s][:], 0.0)

    ghs = persist.tile([128, HEADS * 96], F32, tag="ghs")

    # ---- phase A: chunked pipeline ----
    with (
        tc.tile_pool(name="gps", bufs=1, space="PSUM") as gps,
        tc.tile_pool(name="xp", bufs=2) as xp,
        tc.tile_pool(name="zqkps", bufs=2, space="PSUM") as zqkps,
        tc.tile_pool(name="zvps", bufs=2, space="PSUM") as zvps,
        tc.tile_pool(name="dwqkps", bufs=2, space="PSUM") as dwqkps,
        tc.tile_pool(name="dwvps", bufs=1, space="PSUM") as dwvps,
        tc.tile_pool(name="stp", bufs=2) as stp,
        tc.tile_pool(name="qktp", bufs=2) as qktp,
    ):
        gh = gps.tile([128, HEADS * 96], F32, tag="gh")
        for c in range(NCHUNK):
            s = c % 2
            r0 = max(0, R * c - 1)
            r1 = min(H, R * c + R + 1)
            nrows = r1 - r0
            brow0 = r0 - (R * c - 1)  # z-buf row of image row r0
            r0A = R * c - 1           # zv1p window A abs row 0
            r0B = R * c + 3           # zv1p window B abs row 0

            xil_t = xp.tile([128, 2, nrows * W], FP8, tag="xil")
            xb_lo = xp.tile([128, nrows * W], BF16, tag="xblo")
            xb_hi = xp.tile([64, nrows * W], BF16, tag="xbhi")
            nc.sync.dma_start(
                xil_t[:],
                xil_d[:].rearrange("p (i l) -> p i l", i=2)[:, :, r0 * W : r1 * W],
            )
            nc.sync.dma_start(xb_lo[:], xbf_d[0:128, r0 * W : r1 * W])
            nc.sync.dma_start(xb_hi[:], xbf_d[128:192, r0 * W : r1 * W])

            # conv1 into padded z tiles (groups of <=4 rows)
            for g0 in range(0, nrows, 4):
                gn = min(4, nrows - g0)
                gs = slice(g0 * W, (g0 + gn) * W)
                # q/k: one fp8 DoubleRow matmul per 128-out block (K=192)
                for b in range(3):
                    ps = zqkps.tile([128, 512], F32, tag="zqkps")
                    nc.tensor.matmul(
                        ps[:, 0 : gn * W],
                        w1qk[:, :, b * 128 : (b + 1) * 128],
                        xil_t[:, :, gs],
                        start=True,
                        stop=True,
                        perf_mode=DR,
                    )
                    copy(
                        zqk[s][b][:, brow0 + g0 : brow0 + g0 + gn, 1 : 1 + W],
                        ps[:, 0 : gn * W],
                    )
                # v: bf16, M blocks [0:128] and dup'd [128:256]
                for mb in range(2):
                    ms = slice(mb * 128, (mb + 1) * 128)
                    ps = zvps.tile([128, 512], F32, tag="zvps")
                    nc.tensor.matmul(
                        ps[:, 0 : gn * W], w1v[:, ms], xb_lo[:, gs],
                        start=True, stop=False,
                    )
                    nc.tensor.matmul(
                        ps[:, 0 : gn * W], w1vh[:, ms], xb_hi[:, gs],
                        start=False, stop=True,
                    )
                    if mb == 0:
                        copy(
                            zv0[s][:, brow0 + g0 : brow0 + g0 + gn, 1 : 1 + W],
                            ps[:, 0 : gn * W],
                        )
                    else:
                        # windowed copies: rows 0:64 dup 64:128 hold the same
                        # 64 channels; A window -> parts 0:64, B -> 64:128
                        a0 = r0 + g0
                        a1 = a0 + gn
                        for half, w0 in ((0, r0A), (1, r0B)):
                            lo = max(a0, w0)
                            hi = min(a1, w0 + 6)
                            if lo >= hi:
                                continue
                            p0 = half * 64
                            copy(
                                zv1p[s][p0 : p0 + 64, lo - w0 : hi - w0, 1 : 1 + W],
                                ps[p0 : p0 + 64, (lo - a0) * W : (hi - a0) * W],
                            )

            if c == NCHUNK - 1:
                # bottom halo rows never written this chunk; clear stale data
                for b in range(3):
                    nc.gpsimd.memset(zqk[s][b][:, ZROWS - 1 : ZROWS, :], 0.0)
                nc.gpsimd.memset(zv0[s][:, ZROWS - 1 : ZROWS, :], 0.0)
                nc.gpsimd.memset(zv1p[s][64:128, 5:6, :], 0.0)

            # DW q/k: 5 fp8 DoubleRow matmuls per block per row-group
            st = [stp.tile([128, R, W], BF16, tag=f"st{i}", name=f"st{i}") for i in range(3)]
            for b in range(3):
                zp = zqk[s][b][:]
                for g0o, gn in ((0, 3), (3, 3), (6, 2)):
                    ps = dwqkps.tile([128, 3 * PADW], F32, tag="dwqkps")
                    nrun = gn * PADW - 2
                    for pi, (ta, tb) in enumerate(PAIRS):
                        da = ta[0] * PADW + ta[1]
                        istride = (tb[0] * PADW + tb[1] - da) if tb is not None else 1
                        base = (1 + g0o + ta[0]) * PADW + ta[1] + 1
                        mov = _view(
                            zp,
                            [[ZROWS * PADW, 128], [istride, 2], [1, nrun]],
                            extra_off=base,
                        )
                        nc.tensor.matmul(
                            ps[:, 1 : 1 + nrun], dwstat[b][pi][:], mov,
                            start=(pi == 0), stop=(pi == len(PAIRS) - 1),
                            perf_mode=DR,
                        )
                    psv = ps[:].rearrange("p (r w) -> p r w", w=PADW)
                    copy(st[b][:, g0o : g0o + gn, :], psv[:, 0:gn, 1 : 1 + W])

            # DW v: bf16 diag matmuls
            for g in range(2):
                ps = dwvps.tile([128, 512], F32, tag="dwvps")
                for t, (di, dj) in enumerate(TAPS):
                    nc.tensor.matmul(
                        ps[:],
                        diagv[0][t][:],
                        zv0[s][:, 1 + 4 * g + di : 5 + 4 * g + di, 1 + dj : 1 + dj + W],
                        start=(t == 0), stop=(t == 8),
                    )
                copy(v0[:, c * 1024 + g * 512 : c * 1024 + (g + 1) * 512], ps[:])
            ps = dwvps.tile([128, 512], F32, tag="dwvps")
            for t, (di, dj) in enumerate(TAPS):
                nc.tensor.matmul(
                    ps[:],
                    diagv[1][t][:],
                    zv1p[s][:, 1 + di : 5 + di, 1 + dj : 1 + dj + W],
                    start=(t == 0), stop=(t == 8),
                )
            copy(v1[:, c * 512 : (c + 1) * 512], ps[:])

            # transpose q,k: qkt[:, lt, 0, :] = q^T, [:, lt, 1, :] = k^T
            st_flat = [t_.rearrange("p a b -> p (a b)") for t_ in (s_[:] for s_ in st)]
            qkt = qktp.tile([128, R, 2, 192], BF16, tag="qkt")
            nc.sync.dma_start_transpose(qkt[:, :, 0, 0:128], st_flat[0][:, :])
            nc.scalar.dma_start_transpose(qkt[:, :, 0, 128:192], st_flat[1][0:64, :])
            nc.sync.dma_start_transpose(qkt[:, :, 1, 0:64], st_flat[1][64:128, :])
            nc.scalar.dma_start_transpose(qkt[:, :, 1, 64:192], st_flat[2][:, :])

            # gram accumulation: per (lt, head): q^T[q|k] -> [Gqq|Gqk] at
            # rows 0:48, and k^T k -> Gkk at rows 64:112 (base-64 aligned)
            for lt in range(R):
                first = c == 0 and lt == 0
                last = c == NCHUNK - 1 and lt == R - 1
                for h in range(HEADS):
                    hs = slice(h * DH, (h + 1) * DH)
                    nc.tensor.matmul(
                        gh[0:48, h * 96 : (h + 1) * 96],
                        qkt[:, lt, 0, hs],
                        qkt[:, lt, :, hs],
                        start=(first and h == 0),
                        stop=(last and h == HEADS - 1),
                        skip_group_check=True,
                    )
                    nc.tensor.matmul(
                        gh[64:112, h * 96 : h * 96 + DH],
                        qkt[:, lt, 1, hs],
                        qkt[:, lt, 1, hs],
                        start=(first and h == 0),
                        stop=(last and h == HEADS - 1),
                        skip_group_check=True,
                    )

        nc.vector.tensor_copy(ghs[0:48, :], gh[0:48, :])
        nc.vector.tensor_copy(ghs[64:112, :], gh[64:112, :])

    # ---- phase B ----
    with (
        tc.tile_pool(name="bsb", bufs=1) as bsb,
        tc.tile_pool(name="bps", bufs=1, space="PSUM") as bps,
        tc.tile_pool(name="ops", bufs=4, space="PSUM") as ops,
        tc.tile_pool(name="osb", bufs=4) as osb,
    ):
        attn_bf = bsb.tile([48, HEADS * 48], BF16, tag="attnbf")
        scr = bsb.tile([48, 48], F32, tag="scr")
        scr2 = bsb.tile([128, 48], F32, tag="scr2")
        colv = bsb.tile([48, 1], F32, tag="colv")
        rowv = bsb.tile([1, 48], F32, tag="rowv")
        rkrep = bsb.tile([48, 48], F32, tag="rkrep")
        logits = bsb.tile([48, 48], F32, tag="logits")

        for h in range(HEADS):
            gqq = ghs[0:48, h * 96 : h * 96 + 48]
            gqk = ghs[0:48, h * 96 + 48 : h * 96 + 96]

            # rq_inv = 1/max(sqrt(diag(Gqq)),eps), with 1/sqrt(DH) folded in
            nc.vector.tensor_mul(scr[:], gqq, ident48[:])
            nc.vector.reduce_sum(colv[:], scr[:], axis=mybir.AxisListType.X)
            nc.scalar.activation(colv[:], colv[:], AF.Sqrt)
            nc.vector.tensor_scalar_max(colv[:], colv[:], 1e-12)
            nc.vector.reciprocal(colv[:], colv[:])
            nc.vector.tensor_scalar(
                logits[:],
                gqk,
                colv[:],
                1.0 / math.sqrt(DH),
                op0=mybir.AluOpType.mult,
                op1=mybir.AluOpType.mult,
            )

            # rk_inv: diag(Gkk) via ones-colsum matmul at base-64 partitions,
            # then broadcast along the free (key) dim
            nc.vector.tensor_mul(
                scr2[64:112, :], ghs[64:112, h * 96 : h * 96 + 48], ident48b[64:112, :]
            )
            ps_row = bps.tile([1, 48], F32, tag="psrow")
            nc.tensor.matmul(
                ps_row[:], ones128[64:112, :], scr2[64:112, :], start=True, stop=True
            )
            nc.vector.tensor_copy(rowv[:], ps_row[:])
            nc.scalar.activation(rowv[:], rowv[:], AF.Sqrt)
            nc.vector.tensor_scalar_max(rowv[:], rowv[:], 1e-12)
            nc.vector.reciprocal(rowv[:], rowv[:])
            ps_rep = bps.tile([48, 48], F32, tag="pssmall")
            nc.tensor.matmul(ps_rep[:], ones1x48[:], rowv[:], start=True, stop=True)
            nc.vector.tensor_copy(rkrep[:], ps_rep[:])
            nc.vector.tensor_mul(logits[:], logits[:], rkrep[:])

            # softmax over the free (key) dim
            nc.vector.reduce_max(colv[:], logits[:], axis=mybir.AxisListType.X)
            nc.vector.tensor_scalar_sub(logits[:], logits[:], colv[:])
            nc.scalar.activation(logits[:], logits[:], AF.Exp)
            nc.vector.reduce_sum(colv[:], logits[:], axis=mybir.AxisListType.X)
            nc.vector.reciprocal(colv[:], colv[:])
            nc.vector.tensor_scalar_mul(logits[:], logits[:], colv[:])
            nc.vector.tensor_copy(attn_bf[:, h * 48 : (h + 1) * 48], logits[:])

        # block-diagonal attn (bf16); cols 192:256 duplicate cols 128:192
        bd0 = bsb.tile([128, 256], BF16, tag="bd0")
        bd1 = bsb.tile([64, 256], BF16, tag="bd1")
        nc.gpsimd.memset(bd0[:], 0.0)
        nc.gpsimd.memset(bd1[:], 0.0)
        nc.sync.dma_start(bd0[0:48, 0:48], attn_bf[:, 0:48])
        nc.sync.dma_start(bd0[48:96, 48:96], attn_bf[:, 48:96])
        nc.sync.dma_start(bd0[96:128, 96:144], attn_bf[0:32, 96:144])
        nc.sync.dma_start(bd1[0:16, 96:144], attn_bf[32:48, 96:144])
        nc.sync.dma_start(bd1[16:64, 144:192], attn_bf[:, 144:192])
        nc.sync.dma_start(bd0[96:128, 192:208], attn_bf[0:32, 128:144])
        nc.sync.dma_start(bd1[0:16, 192:208], attn_bf[32:48, 128:144])
        nc.sync.dma_start(bd1[16:64, 208:256], attn_bf[:, 144:192])

        # W_effT = BD(attn).T @ W_outT; rows 128:192 duplicated into 64:128
        # of weff1 so odd v1 spatial groups can use base-64 stationaries
        weff0 = bsb.tile([128, 256], BF16, tag="weff0")
        weff1 = bsb.tile([128, 256], BF16, tag="weff1")
        nc.gpsimd.memset(weff0[:], 0.0)
        nc.gpsimd.memset(weff1[:], 0.0)
        for m0, m1, wt in [(0, 128, weff0), (128, 256, weff1)]:
            pw = bps.tile([128, C], F32, tag="pweff")
            nc.tensor.matmul(pw[0 : m1 - m0, :], bd0[:, m0:m1], woutt0_bf[:], start=True, stop=False)
            nc.tensor.matmul(pw[0 : m1 - m0, :], bd1[:, m0:m1], woutt1_bf[:], start=False, stop=True)
            copy(wt[0 : m1 - m0, 0:C], pw[0 : m1 - m0, :])

        # y = W_effT.T @ v
        for g in range(L // 512):
            sl = slice(g * 512, (g + 1) * 512)
            half = g % 2
            v1sl = v1[64 * half : 64 * half + 64, (g // 2) * 512 : (g // 2 + 1) * 512]
            for m0, m1 in [(0, 128), (128, 192)]:
                po = ops.tile([128, 512], F32, tag="ops")
                nc.tensor.matmul(po[:, :], weff0[:, m0 : m0 + 128], v0[:, sl], start=True, stop=False)
                nc.tensor.matmul(
                    po[:, :],
                    weff1[64 * half : 64 * half + 64, m0 : m0 + 128],
                    v1sl,
                    start=False,
                    stop=True,
                )
                ot = osb.tile([m1 - m0, 512], F32, tag=f"o{m0}", name=f"o{m0}")
                copy(ot[:], po[0 : m1 - m0, :])
                nc.sync.dma_start(y_d[m0:m1, sl], ot[:])


_NC_CACHE = None


def _get_nc():
    global _NC_CACHE
    if _NC_CACHE is None:
        _NC_CACHE = build_nc()
    return _NC_CACHE


def _prep_weights(w_proj1, w_dw, w_out):
    import ml_dtypes

    w1 = np.asarray(w_proj1, np.float32).reshape(CQKV, C)  # [out 576, in 192]
    # q/k 1x1: fp8 interleaved stationary [128, 2, 384]
    w1qk = np.zeros((128, 2, 384), np.float32)
    w1qk[:, 0, :] = w1[0:384, 0:128].T
    w1qk[0:64, 1, :] = w1[0:384, 128:192].T
    w1qk = np.ascontiguousarray(w1qk.reshape(128, 2 * 384)).astype(
        ml_dtypes.float8_e4m3
    )
    # v 1x1: bf16 [192, 256] with cols 192:256 duplicating cols 128:192
    w1v = np.zeros((C, 256), np.float32)
    w1v[:, 0:128] = w1[384:512, :].T
    w1v[:, 128:192] = w1[512:576, :].T
    w1v[:, 192:256] = w1[512:576, :].T
    w1v = np.ascontiguousarray(w1v).astype(ml_dtypes.bfloat16)
    # dw weights [640, 9]: qk 0:384, v0 384:512, v1 duplicated pair 512:640
    wdwf = np.asarray(w_dw, np.float32).reshape(CQKV, 9)
    wdw = np.zeros((640, 9), np.float32)
    wdw[0:512] = wdwf[0:512]
    wdw[512:576] = wdwf[512:576]
    wdw[576:640] = wdwf[512:576]
    # output projection, transposed, bf16
    woutt = (
        np.ascontiguousarray(np.asarray(w_out, np.float32).reshape(C, C).T)
        .astype(ml_dtypes.bfloat16)
    )
    return w1qk, w1v, wdw, woutt


def kernel(x, w_proj1, w_dw, pos_emb, w_out, _trace=False):
    from concourse.bass_utils import run_bass_kernel_spmd

    import ml_dtypes

    xf = np.asarray(x, dtype=np.float32).reshape(N_CORES, C, L)
    xbf = xf.astype(ml_dtypes.bfloat16)
    x8 = xf.astype(ml_dtypes.float8_e4m3)
    xil = np.zeros((N_CORES, 128, 2, L), ml_dtypes.float8_e4m3)
    xil[:, :, 0, :] = x8[:, 0:128]
    xil[:, 0:64, 1, :] = x8[:, 128:192]
    xil = xil.reshape(N_CORES, 128, 2 * L)

    w1qk, w1v, wdw, woutt = _prep_weights(w_proj1, w_dw, w_out)
    # pos_emb adds a per-head constant to every logit in its softmax row;
    # softmax is shift-invariant, so it has no effect on the output.

    nc = _get_nc()
    in_maps = [
        {
            "xil": np.ascontiguousarray(xil[b]),
            "xbf": np.ascontiguousarray(xbf[b]),
            "w1qk": w1qk,
            "w1v": w1v,
            "wdw": wdw,
            "woutt": woutt,
        }
        for b in range(N_CORES)
    ]
    res = run_bass_kernel_spmd(nc, in_maps, list(range(N_CORES)), trace=_trace)
    out = np.stack([res.results[b]["y"].reshape(C, H, W) for b in range(N_CORES)])
    if _trace:
        kernel.last_exec_time_ns = res.exec_time_ns
        kernel.last_profile = res
    return out.astype(np.float32)
